# revision 5
# baseline (speedup 1.0000x reference)
"""Multi-head attention (B=4, S=2048, D=1024, H=16) on 8 trn2 NeuronCores, v2.

Sharding: core c -> batch b = c//2, head-group hg = c%2 (8 heads, 512 dims).

v2 changes vs baseline:
  * bf16 operands for all matmuls (halves DMA + weight-load time).
  * Q/K projections in fp8e4m3 with MatmulPerfMode.DoubleRow (K=256 per
    matmul -> 2.1x measured PE throughput on these GEMMs).
  * exp split between ACT (exact, 11/16 k-tiles) and DVE (Schraudolph
    bit-trick exp -> bf16 bit pattern via int16 convert, 5/16 k-tiles) so the
    scalar engine stops gating the attention inner loop.
  * output stored bf16, upcast + bias on host.
"""

import numpy as np

B, S, D = 4, 2048, 1024
H, DK = 16, 64
NCORES = 8
DS = 512          # feature dims per core (8 heads)
FCH = 8           # 128-wide feature chunks in D
C2 = 4            # 256-wide feature chunks in D (DoubleRow pairs)
DT = 4            # d-tiles (head pairs) per core
QB = 4            # q blocks of 512
KT = 16           # k tiles of 128
TT = 16           # token tiles of 128

# Schraudolph exp -> bf16 bits:  bits = x*(128/ln2) + 127*128 + c
SCH_A = 128.0 / np.log(2.0)
SCH_C = -5.51 + 0.5          # minimax offset; +0.5 compensates truncation
SCH_B = 127.0 * 128.0 + SCH_C

import os as _os

QK_FP8 = _os.environ.get("QK_FP8", "0") == "1"
XACT = int(_os.environ.get("XACT", "624"))  # exp cols on ACT; rest on DVE
KTX = int(_os.environ.get("KTX", "16"))     # k-tiles per block (timing bisects)
NOEXP = int(_os.environ.get("NOEXP", "0"))  # 1: ctx reads const; 2: + exp still runs
NOOUT = _os.environ.get("NOOUT", "0") == "1"   # timing bisect: skip out-proj
DMAONLY = _os.environ.get("DMAONLY", "0") == "1"  # timing bisect: loads only

_cache = {}


def _build_nc(niter=1):
    import concourse.bass as bass  # noqa: F401
    import concourse.mybir as mybir
    from concourse import bacc
    from concourse.tile import TileContext
    from contextlib import nullcontext

    f32 = mybir.dt.float32
    bf16 = mybir.dt.bfloat16
    i16 = mybir.dt.int16
    fp8 = mybir.dt.float8e4
    EXP = mybir.ActivationFunctionType.Exp
    DR = mybir.MatmulPerfMode.DoubleRow
    MULT = mybir.AluOpType.mult
    ADD = mybir.AluOpType.add

    nc = bacc.Bacc(None, target_bir_lowering=False)
    qk_dt = fp8 if QK_FP8 else bf16
    xq_in = nc.declare_dram_parameter("xq", [128, C2, 2, S], qk_dt, isOutput=False)
    xk_in = nc.declare_dram_parameter("xk", [128, C2, 2, S], qk_dt, isOutput=False)
    vt_in = nc.declare_dram_parameter("vt", [128, FCH, S], bf16, isOutput=False)
    wq_in = nc.declare_dram_parameter("wq", [128, C2, 2, DS], qk_dt, isOutput=False)
    wk_in = nc.declare_dram_parameter("wk", [128, C2, 2, DS], qk_dt, isOutput=False)
    wv_in = nc.declare_dram_parameter("wv", [128, FCH, 520], bf16, isOutput=False)
    wo_in = nc.declare_dram_parameter("wo", [128, DT, D], bf16, isOutput=False)
    bq_in = nc.declare_dram_parameter("bq", [128, DT], f32, isOutput=False)
    bk_in = nc.declare_dram_parameter("bk", [128, DT], f32, isOutput=False)
    bvr_in = nc.declare_dram_parameter("bvr", [128, 520], f32, isOutput=False)
    mb_in = nc.declare_dram_parameter("mb", [128, KT], f32, isOutput=False)
    mbs_in = nc.declare_dram_parameter("mbs", [128, KT], f32, isOutput=False)
    out_d = nc.declare_dram_parameter("out", [S, D], bf16, isOutput=True)
    rscr = nc.dram_tensor("rscr", [DT * QB * 2, 512], bf16)

    with TileContext(nc) as tc:
        with (
            tc.For_i(0, niter, 1) if niter > 1 else nullcontext(),
            tc.tile_pool(name="keep", bufs=1) as keep,
            tc.tile_pool(name="sc", bufs=3, space="PSUM") as pssc,
            tc.tile_pool(name="acc", bufs=2, space="PSUM") as psacc,
        ):
            # ---- small constants (DMAs issued after the big loads below) ----
            bq_sb = keep.tile([128, DT], f32)
            bk_sb = keep.tile([128, DT], f32)
            bvr_sb = keep.tile([128, 520], f32)
            mb_sb = keep.tile([128, KT], f32)
            mbs_sb = keep.tile([128, KT], f32)

            qt_sb = [keep.tile([128, S], bf16, tag="qt", bufs=DT, name=f"qt{t}") for t in range(DT)]
            kt_sb = [keep.tile([128, S], bf16, tag="kt", bufs=DT, name=f"kt{t}") for t in range(DT)]
            v_sb = [keep.tile([128, 520], bf16, tag="v", bufs=TT, name=f"v{t}") for t in range(TT)]

            with tc.tile_pool(name="proj", bufs=1) as proj:
                # ---- V projection first (context matmuls need all of V) ----
                wv_sb = proj.tile([128, FCH, 520], bf16, tag="wv", bufs=1)
                nc.sync.dma_start(out=wv_sb, in_=wv_in[:, :, :])
                vc = []
                for c in range(FCH):
                    v_t = proj.tile([128, S], bf16, tag="xt", bufs=FCH, name=f"xv{c}")
                    nc.sync.dma_start(out=v_t, in_=vt_in[:, c, :])
                    vc.append(v_t)
                nc.sync.dma_start(out=bvr_sb, in_=bvr_in[:, :])
                nc.sync.dma_start(out=bq_sb, in_=bq_in[:, :])
                nc.sync.dma_start(out=bk_sb, in_=bk_in[:, :])
                nc.sync.dma_start(out=mb_sb, in_=mb_in[:, :])
                nc.sync.dma_start(out=mbs_sb, in_=mbs_in[:, :])
                for tt in range(TT):
                    vps = pssc.tile([128, 520], f32, tag="sc", name=f"vps{tt}")
                    for c in range(FCH):
                        nc.tensor.matmul(
                            vps[:, 0:512], vc[c][:, tt * 128:(tt + 1) * 128],
                            wv_sb[:, c, 0:512],
                            start=(c == 0), stop=(c == FCH - 1),
                        )
                        nc.tensor.matmul(
                            vps[:, 512:520], vc[c][:, tt * 128:(tt + 1) * 128],
                            wv_sb[:, c, 512:520],
                            start=(c == 0), stop=(c == FCH - 1),
                        )
                    nc.vector.tensor_add(v_sb[tt], vps, bvr_sb)

                # ---- Q^T / K^T projections, fp8 DoubleRow (K=256/mm) ----
                for name, w_dram, x_dram, b_sb, o_tiles in (
                    ("k", wk_in, xk_in, bk_sb, kt_sb),
                    ("q", wq_in, xq_in, bq_sb, qt_sb),
                ):
                    w_sb = proj.tile([128, C2, 2, DS], qk_dt, tag="w8", bufs=2, name=f"w{name}")
                    nc.sync.dma_start(out=w_sb, in_=w_dram[:, :, :, :])
                    xc = []
                    for c in range(C2):
                        x_t = proj.tile([128, 2, S], qk_dt, tag="x8", bufs=2 * C2, name=f"x{name}{c}")
                        nc.sync.dma_start(out=x_t, in_=x_dram[:, c, :, :])
                        xc.append(x_t)
                    for t in range(DT):
                        for qb in range(QB):
                            acc = psacc.tile([128, 512], f32, tag="acc",
                                             name=f"pa{name}{t}{qb}")
                            for c in range(C2):
                                if QK_FP8:
                                    nc.tensor.matmul(
                                        acc,
                                        w_sb[:, c, :, t * 128:(t + 1) * 128],
                                        xc[c][:, :, qb * 512:(qb + 1) * 512],
                                        start=(c == 0), stop=(c == C2 - 1),
                                        perf_mode=DR,
                                    )
                                else:
                                    for o in range(2):
                                        nc.tensor.matmul(
                                            acc,
                                            w_sb[:, c, o, t * 128:(t + 1) * 128],
                                            xc[c][:, o, qb * 512:(qb + 1) * 512],
                                            start=(c == 0 and o == 0),
                                            stop=(c == C2 - 1 and o == 1),
                                        )
                            nc.vector.tensor_scalar_add(
                                o_tiles[t][:, qb * 512:(qb + 1) * 512],
                                acc,
                                b_sb[:, t:t + 1],
                            )

            # ---- attention, per head-pair and q-block ----
            with tc.tile_pool(name="attn", bufs=1) as attn:
                wo_sb = attn.tile([128, DT, D], bf16, tag="wo", bufs=1)
                nc.sync.dma_start(out=wo_sb, in_=wo_in[:, :, :])
                cn_sb = [attn.tile([128, S], bf16, tag="cn", bufs=DT, name=f"cn{h}") for h in range(DT)]
                if NOEXP:
                    ec = attn.tile([128, 1024], bf16, tag="ec", bufs=1)
                    nc.vector.memset(ec, 1.0)

                # unnormalized context (+denominator row 64) parked in SBUF so
                # the PSUM acc pair frees fast -> sct can triple-buffer
                cu_sb = [
                    attn.tile([65, 512], bf16, tag="cu", bufs=2 * DT * QB, name=f"cu{i}")
                    for i in range(2 * DT * QB)
                ]

                blocks = [(hp, qb) for hp in range(DT) for qb in range(QB)]
                NB = len(blocks) if KTX else 0
                NG = NB * KTX
                LOOK = 2
                bstate = {}

                def emit_sc(g):
                    hp, qb = blocks[g // KTX]
                    k = g % KTX
                    sct = pssc.tile([128, 1024], f32, tag="sc", name=f"sct{g}")
                    nc.tensor.matmul(
                        sct[:, 0:512],
                        kt_sb[hp][0:64, k * 128:(k + 1) * 128],
                        qt_sb[hp][0:64, qb * 512:(qb + 1) * 512],
                        start=True, stop=True, tile_position=(0, 0),
                    )
                    nc.tensor.matmul(
                        sct[:, 512:1024],
                        kt_sb[hp][64:128, k * 128:(k + 1) * 128],
                        qt_sb[hp][64:128, qb * 512:(qb + 1) * 512],
                        start=True, stop=True, tile_position=(64, 0),
                    )
                    if NOEXP:
                        et = ec
                        if NOEXP >= 2:
                            src_a = sct[:, 0:XACT] if NOEXP == 2 else ec[:, 0:XACT]
                            src_d = sct[:, XACT:1024] if NOEXP == 2 else bvr_sb[:, 0:1024 - XACT]
                            dt_ = attn.tile([128, 1024], bf16, tag="et", bufs=8, name=f"dt{g}")
                            nc.scalar.activation(
                                out=dt_[:, 0:XACT], in_=src_a, func=EXP,
                                bias=mb_sb[:, k:k + 1], scale=0.125,
                            )
                            nc.vector.tensor_scalar(
                                out=dt_[:, XACT:1024].bitcast(i16), in0=src_d,
                                scalar1=float(SCH_A * 0.125),
                                scalar2=mbs_sb[:, k:k + 1], op0=MULT, op1=ADD,
                            )
                    else:
                        et = attn.tile([128, 1024], bf16, tag="et", bufs=8, name=f"et{g}")
                        nc.scalar.activation(
                            out=et[:, 0:XACT], in_=sct[:, 0:XACT], func=EXP,
                            bias=mb_sb[:, k:k + 1], scale=0.125,
                        )
                        if XACT < 1024:
                            nc.vector.tensor_scalar(
                                out=et[:, XACT:1024].bitcast(i16),
                                in0=sct[:, XACT:1024],
                                scalar1=float(SCH_A * 0.125),
                                scalar2=mbs_sb[:, k:k + 1], op0=MULT, op1=ADD,
                            )
                    bstate[g] = et

                def emit_ctx(g):
                    bi = g // KTX
                    hp, qb = blocks[bi]
                    k = g % KTX
                    if k == 0:
                        bstate["acc", bi] = (
                            psacc.tile([65, 512], f32, tag="acc", name=f"ca0_{bi}"),
                            psacc.tile([65, 512], f32, tag="acc", name=f"ca1_{bi}"),
                        )
                    acc0, acc1 = bstate["acc", bi]
                    et = bstate.pop(g)
                    lh0, lh1 = 2 * hp, 2 * hp + 1
                    nc.tensor.matmul(
                        acc0, v_sb[k][:, lh0 * 65:lh0 * 65 + 65], et[:, 0:512],
                        start=(k == 0), stop=(k == KTX - 1),
                    )
                    nc.tensor.matmul(
                        acc1, v_sb[k][:, lh1 * 65:lh1 * 65 + 65], et[:, 512:1024],
                        start=(k == 0), stop=(k == KTX - 1),
                    )
                    if k == KTX - 1:
                        # park unnormalized ctx in SBUF: h0 copy on ACT, h1 on DVE
                        nc.scalar.copy(cu_sb[2 * bi], acc0)
                        nc.vector.tensor_copy(cu_sb[2 * bi + 1], acc1)
                        if qb == 0:
                            # prefetch this block's normalize so the output
                            # projection can start without a pipeline bubble
                            emit_norm(bi)

                def emit_norm(bi):
                    hp, qb = blocks[bi]
                    for half in range(2):
                        idx = 2 * bi + half
                        cu = cu_sb[idx]
                        r1 = attn.tile([128, 512], bf16, tag="r1", bufs=4, name=f"r1_{idx}")
                        with nc.allow_low_precision(reason="softmax denom recip in bf16"):
                            nc.vector.reciprocal(r1[64:65, :], cu[64:65, :])
                        nc.sync.dma_start(out=rscr[idx, :], in_=r1[64:65, :])
                        rr = attn.tile([64, 512], bf16, tag="rr", bufs=4, name=f"rr_{idx}")
                        nc.sync.dma_start(
                            out=rr, in_=rscr[idx, :].unsqueeze(0).partition_broadcast(64)
                        )
                        if half == 0:
                            nc.vector.tensor_mul(
                                cn_sb[hp][0:64, qb * 512:(qb + 1) * 512], cu[0:64, :], rr
                            )
                        else:
                            tm = attn.tile([64, 512], bf16, tag="tm", bufs=4, name=f"tm_{idx}")
                            nc.vector.tensor_mul(tm, cu[0:64, :], rr)
                            nc.sync.dma_start(
                                out=cn_sb[hp][64:128, qb * 512:(qb + 1) * 512], in_=tm
                            )


                for g in range(NG + LOOK):
                    # at block boundaries, retire the block's last ctx before
                    # issuing the next scores so the acc-pair copies start
                    # (and free the PSUM pair) as early as possible
                    late = g >= LOOK and (g - LOOK) % KTX == KTX - 1
                    if late:
                        emit_ctx(g - LOOK)
                    if g < NG:
                        emit_sc(g)
                    if g >= LOOK and not late:
                        emit_ctx(g - LOOK)

                # ---- normalize (pipelined ahead of output projection) ----
                # ---- output projection, qb-pipelined after its normalizes ----
                if not NOOUT and KTX:
                    for qb in range(QB + 1):
                        if 0 < qb < QB:
                            for hp in range(DT):
                                emit_norm(hp * QB + qb)
                        if qb == 0:
                            continue
                        for qt_i in range(4 * (qb - 1), 4 * qb):
                            pos = [
                                psacc.tile([128, 512], f32, tag="acc", name=f"po{qt_i}{nb}")
                                for nb in range(2)
                            ]
                            for hp in range(DT):
                                for nb in range(2):
                                    nc.tensor.matmul(
                                        pos[nb],
                                        cn_sb[hp][:, qt_i * 128:(qt_i + 1) * 128],
                                        wo_sb[:, hp, nb * 512:(nb + 1) * 512],
                                        start=(hp == 0), stop=(hp == DT - 1),
                                    )
                            for nb in range(2):
                                os_t = attn.tile([128, 512], bf16, tag="os", bufs=4, name=f"os{qt_i}{nb}")
                                nc.scalar.copy(os_t, pos[nb])
                                nc.sync.dma_start(
                                    out=out_d[qt_i * 128:(qt_i + 1) * 128, nb * 512:(nb + 1) * 512],
                                    in_=os_t,
                                )

    nc.finalize()
    return nc


def _build_dma_nc(niter=1):
    import concourse.mybir as mybir
    from concourse import bacc
    from concourse.tile import TileContext
    from contextlib import nullcontext

    f32 = mybir.dt.float32
    bf16 = mybir.dt.bfloat16
    fp8 = mybir.dt.float8e4
    qk_dt = fp8 if QK_FP8 else bf16

    nc = bacc.Bacc(None, target_bir_lowering=False)
    xq_in = nc.declare_dram_parameter("xq", [128, C2, 2, S], qk_dt, isOutput=False)
    xk_in = nc.declare_dram_parameter("xk", [128, C2, 2, S], qk_dt, isOutput=False)
    vt_in = nc.declare_dram_parameter("vt", [128, FCH, S], bf16, isOutput=False)
    wq_in = nc.declare_dram_parameter("wq", [128, C2, 2, DS], qk_dt, isOutput=False)
    wk_in = nc.declare_dram_parameter("wk", [128, C2, 2, DS], qk_dt, isOutput=False)
    wv_in = nc.declare_dram_parameter("wv", [128, FCH, 520], bf16, isOutput=False)
    wo_in = nc.declare_dram_parameter("wo", [128, DT, D], bf16, isOutput=False)
    out_d = nc.declare_dram_parameter("out", [S, D], bf16, isOutput=True)

    with TileContext(nc) as tc:
        with (
            tc.For_i(0, niter, 1) if niter > 1 else nullcontext(),
            tc.tile_pool(name="keep", bufs=1) as keep,
            tc.tile_pool(name="dps", bufs=1, space="PSUM") as dps,
        ):
            big = keep.tile([128, C2, 2, S], qk_dt, tag="b8", bufs=2, name="bg1")
            big2 = keep.tile([128, C2, 2, S], qk_dt, tag="b8", bufs=2, name="bg2")
            vtl = keep.tile([128, FCH, S], bf16)
            wvl = keep.tile([128, FCH, 520], bf16)
            wol = keep.tile([128, DT, D], bf16)
            wql = keep.tile([128, C2, 2, DS], qk_dt, tag="w8", bufs=2, name="wg1")
            wkl = keep.tile([128, C2, 2, DS], qk_dt, tag="w8", bufs=2, name="wg2")
            nc.sync.dma_start(out=big, in_=xq_in[:, :, :, :])
            nc.sync.dma_start(out=big2, in_=xk_in[:, :, :, :])
            nc.sync.dma_start(out=vtl, in_=vt_in[:, :, :])
            nc.sync.dma_start(out=wvl, in_=wv_in[:, :, :])
            nc.sync.dma_start(out=wol, in_=wo_in[:, :, :])
            nc.sync.dma_start(out=wql, in_=wq_in[:, :, :, :])
            nc.sync.dma_start(out=wkl, in_=wk_in[:, :, :, :])
            pt = dps.tile([128, 512], f32)
            nc.tensor.matmul(pt, vtl[:, 0, 0:128], wvl[:, 0, 0:512],
                             start=True, stop=True)
            nc.tensor.matmul(pt, big.bitcast(bf16)[:, 0, 0, 0:128],
                             big2.bitcast(bf16)[:, 0, 0, 0:512],
                             start=True, stop=True)
            nc.tensor.matmul(pt, wql.bitcast(bf16)[:, 0, 0, 0:128],
                             wkl.bitcast(bf16)[:, 0, 0, 0:512],
                             start=True, stop=True)
            nc.tensor.matmul(pt, wol[:, 0, 0:128], wol[:, 0, 0:512],
                             start=True, stop=True)
            ot = keep.tile([128, 512], bf16)
            nc.vector.tensor_copy(ot, pt)
            nc.sync.dma_start(out=out_d[0:128, 0:512], in_=ot)
    nc.finalize()
    return nc


def _get_nc(niter=1):
    key = ("nc", niter, DMAONLY)
    if key not in _cache:
        _cache[key] = (_build_dma_nc if DMAONLY else _build_nc)(niter)
    return _cache[key]


def _make_in_maps(query, key, value, mask, Wq, bq, Wk, bk, Wv, bv, Wo, bo):
    import ml_dtypes

    f = np.float32
    bf = ml_dtypes.bfloat16
    f8 = ml_dtypes.float8_e4m3fn
    in_maps = []
    for c in range(NCORES):
        b, hg = c // 2, c % 2
        hs = hg * DS
        wv_aug = np.zeros((D, 520), f)
        bvr_row = np.zeros((520,), f)
        for lh in range(8):
            wv_aug[:, lh * 65:lh * 65 + 64] = Wv[:, hs + lh * 64: hs + (lh + 1) * 64]
            bvr_row[lh * 65:lh * 65 + 64] = bv[hs + lh * 64: hs + (lh + 1) * 64]
            bvr_row[lh * 65 + 64] = 1.0
        mbias = np.where(mask[b, 0, 0, :] == 0, f(-1e9), f(0.0)).astype(f)

        qk_np = f8 if QK_FP8 else bf

        def dr_x(x):  # [S, D] -> [128, C2, 2, S] pairs
            xt = np.ascontiguousarray(x.T, dtype=f)           # [D, S]
            return np.ascontiguousarray(
                xt.reshape(C2, 2, 128, S).transpose(2, 0, 1, 3)
            ).astype(qk_np)

        def dr_w(W):  # [D, DS slice] -> [128, C2, 2, DS] pairs
            Ws = np.ascontiguousarray(W[:, hs:hs + DS], dtype=f)
            return np.ascontiguousarray(
                Ws.reshape(C2, 2, 128, DS).transpose(2, 0, 1, 3)
            ).astype(qk_np)

        vt = np.ascontiguousarray(value[b].T, dtype=f)        # [D, S]
        in_maps.append({
            "xq": dr_x(query[b]),
            "xk": dr_x(key[b]),
            "vt": np.ascontiguousarray(vt.reshape(FCH, 128, S).transpose(1, 0, 2)).astype(bf),
            "wq": dr_w(Wq),
            "wk": dr_w(Wk),
            "wv": np.ascontiguousarray(
                wv_aug.reshape(FCH, 128, 520).transpose(1, 0, 2)
            ).astype(bf),
            "wo": np.ascontiguousarray(
                np.asarray(Wo, f)[hs:hs + DS, :].reshape(DT, 128, D).transpose(1, 0, 2)
            ).astype(bf),
            "bq": np.ascontiguousarray(np.asarray(bq, f)[hs:hs + DS].reshape(DT, 128).T),
            "bk": np.ascontiguousarray(np.asarray(bk, f)[hs:hs + DS].reshape(DT, 128).T),
            "bvr": np.tile(bvr_row[None, :], (128, 1)).astype(f),
            "mb": np.ascontiguousarray(mbias.reshape(KT, 128).T, dtype=f),
            "mbs": np.ascontiguousarray(
                (mbias * f(SCH_A) + f(SCH_B)).reshape(KT, 128).T, dtype=f
            ),
        })
    return in_maps


def kernel(query, key, value, mask, Wq, bq, Wk, bk, Wv, bv, Wo, bo):
    from concourse.bass_utils import run_bass_kernel_spmd

    args = [np.asarray(a) for a in (query, key, value, mask, Wq, bq, Wk, bk, Wv, bv, Wo, bo)]
    query, key, value, mask, Wq, bq, Wk, bk, Wv, bv, Wo, bo = args
    nc = _get_nc()
    in_maps = _make_in_maps(query, key, value, mask, Wq, bq, Wk, bk, Wv, bv, Wo, bo)
    res = run_bass_kernel_spmd(nc, in_maps, list(range(NCORES)))
    out = np.empty((B, S, D), np.float32)
    for b in range(B):
        out[b] = (
            res.results[2 * b]["out"].astype(np.float32)
            + res.results[2 * b + 1]["out"].astype(np.float32)
            + bo[None, :]
        )
    return out


# revision 6
# speedup vs baseline: 1.0594x; 1.0594x over previous
"""Multi-head attention (B=4, S=2048, D=1024, H=16) on 8 trn2 NeuronCores, v2.

Sharding: core c -> batch b = c//2, head-group hg = c%2 (8 heads, 512 dims).

v2 changes vs baseline:
  * bf16 operands for all matmuls (halves DMA + weight-load time).
  * Q/K projections in fp8e4m3 with MatmulPerfMode.DoubleRow (K=256 per
    matmul -> 2.1x measured PE throughput on these GEMMs).
  * exp split between ACT (exact, 11/16 k-tiles) and DVE (Schraudolph
    bit-trick exp -> bf16 bit pattern via int16 convert, 5/16 k-tiles) so the
    scalar engine stops gating the attention inner loop.
  * output stored bf16, upcast + bias on host.
"""

import numpy as np

B, S, D = 4, 2048, 1024
H, DK = 16, 64
NCORES = 8
DS = 512          # feature dims per core (8 heads)
FCH = 8           # 128-wide feature chunks in D
C2 = 4            # 256-wide feature chunks in D (DoubleRow pairs)
DT = 4            # d-tiles (head pairs) per core
QB = 4            # q blocks of 512
KT = 16           # k tiles of 128
TT = 16           # token tiles of 128

# Schraudolph exp -> bf16 bits:  bits = x*(128/ln2) + 127*128 + c
SCH_A = 128.0 / np.log(2.0)
SCH_C = -5.51 + 0.5          # minimax offset; +0.5 compensates truncation
SCH_B = 127.0 * 128.0 + SCH_C

import os as _os

QK_FP8 = _os.environ.get("QK_FP8", "0") == "1"
XACT = int(_os.environ.get("XACT", "624"))  # exp cols on ACT; rest on DVE
KTX = int(_os.environ.get("KTX", "16"))     # k-tiles per block (timing bisects)
NOEXP = int(_os.environ.get("NOEXP", "0"))  # 1: ctx reads const; 2: + exp still runs
NOOUT = _os.environ.get("NOOUT", "0") == "1"   # timing bisect: skip out-proj
DMAONLY = _os.environ.get("DMAONLY", "0") == "1"  # timing bisect: loads only

_cache = {}


def _build_nc(niter=1):
    import concourse.bass as bass  # noqa: F401
    import concourse.mybir as mybir
    from concourse import bacc
    from concourse.tile import TileContext
    from contextlib import nullcontext

    f32 = mybir.dt.float32
    bf16 = mybir.dt.bfloat16
    i16 = mybir.dt.int16
    fp8 = mybir.dt.float8e4
    EXP = mybir.ActivationFunctionType.Exp
    DR = mybir.MatmulPerfMode.DoubleRow
    MULT = mybir.AluOpType.mult
    ADD = mybir.AluOpType.add

    nc = bacc.Bacc(None, target_bir_lowering=False)
    qk_dt = fp8 if QK_FP8 else bf16
    xq_in = nc.declare_dram_parameter("xq", [128, C2, 2, S], qk_dt, isOutput=False)
    xk_in = nc.declare_dram_parameter("xk", [128, C2, 2, S], qk_dt, isOutput=False)
    vt_in = nc.declare_dram_parameter("vt", [128, FCH, S], bf16, isOutput=False)
    wq_in = nc.declare_dram_parameter("wq", [128, C2, 2, DS], qk_dt, isOutput=False)
    wk_in = nc.declare_dram_parameter("wk", [128, C2, 2, DS], qk_dt, isOutput=False)
    wv_in = nc.declare_dram_parameter("wv", [128, FCH, 520], bf16, isOutput=False)
    wo_in = nc.declare_dram_parameter("wo", [128, DT, D], bf16, isOutput=False)
    bq_in = nc.declare_dram_parameter("bq", [128, DT], f32, isOutput=False)
    bk_in = nc.declare_dram_parameter("bk", [128, DT], f32, isOutput=False)
    bvr_in = nc.declare_dram_parameter("bvr", [128, 520], f32, isOutput=False)
    mb_in = nc.declare_dram_parameter("mb", [128, KT], f32, isOutput=False)
    mbs_in = nc.declare_dram_parameter("mbs", [128, KT], f32, isOutput=False)
    out_d = nc.declare_dram_parameter("out", [S, D], bf16, isOutput=True)
    rscr = nc.dram_tensor("rscr", [DT * QB * 2, 512], bf16)

    with TileContext(nc) as tc:
        with (
            tc.For_i(0, niter, 1) if niter > 1 else nullcontext(),
            tc.tile_pool(name="keep", bufs=1) as keep,
            tc.tile_pool(name="sc", bufs=3, space="PSUM") as pssc,
            tc.tile_pool(name="acc", bufs=2, space="PSUM") as psacc,
        ):
            # ---- small constants (DMAs issued after the big loads below) ----
            bq_sb = keep.tile([128, DT], f32)
            bk_sb = keep.tile([128, DT], f32)
            bvr_sb = keep.tile([128, 520], f32)
            mb_sb = keep.tile([128, KT], f32)
            mbs_sb = keep.tile([128, KT], f32)

            qt_sb = [keep.tile([128, S], bf16, tag="qt", bufs=DT, name=f"qt{t}") for t in range(DT)]
            kt_sb = [keep.tile([128, S], bf16, tag="kt", bufs=DT, name=f"kt{t}") for t in range(DT)]
            v_sb = [keep.tile([128, 520], bf16, tag="v", bufs=TT, name=f"v{t}") for t in range(TT)]

            with tc.tile_pool(name="proj", bufs=1) as proj:
                # ---- V projection first (context matmuls need all of V) ----
                wv_sb = proj.tile([128, FCH, 520], bf16, tag="wv", bufs=1)
                nc.sync.dma_start(out=wv_sb, in_=wv_in[:, :, :])
                vc = []
                for c in range(FCH):
                    v_t = proj.tile([128, S], bf16, tag="xt", bufs=FCH, name=f"xv{c}")
                    nc.sync.dma_start(out=v_t, in_=vt_in[:, c, :])
                    vc.append(v_t)
                nc.sync.dma_start(out=bvr_sb, in_=bvr_in[:, :])
                nc.sync.dma_start(out=bq_sb, in_=bq_in[:, :])
                nc.sync.dma_start(out=bk_sb, in_=bk_in[:, :])
                nc.sync.dma_start(out=mb_sb, in_=mb_in[:, :])
                nc.sync.dma_start(out=mbs_sb, in_=mbs_in[:, :])
                for tt in range(TT):
                    vps = pssc.tile([128, 520], f32, tag="sc", name=f"vps{tt}")
                    for c in range(FCH):
                        nc.tensor.matmul(
                            vps[:, 0:512], vc[c][:, tt * 128:(tt + 1) * 128],
                            wv_sb[:, c, 0:512],
                            start=(c == 0), stop=(c == FCH - 1),
                        )
                        nc.tensor.matmul(
                            vps[:, 512:520], vc[c][:, tt * 128:(tt + 1) * 128],
                            wv_sb[:, c, 512:520],
                            start=(c == 0), stop=(c == FCH - 1),
                        )
                    nc.vector.tensor_add(v_sb[tt], vps, bvr_sb)

                # ---- Q^T / K^T projections, fp8 DoubleRow (K=256/mm) ----
                for name, w_dram, x_dram, b_sb, o_tiles in (
                    ("k", wk_in, xk_in, bk_sb, kt_sb),
                    ("q", wq_in, xq_in, bq_sb, qt_sb),
                ):
                    w_sb = proj.tile([128, C2, 2, DS], qk_dt, tag="w8", bufs=2, name=f"w{name}")
                    nc.sync.dma_start(out=w_sb, in_=w_dram[:, :, :, :])
                    xc = []
                    for c in range(C2):
                        x_t = proj.tile([128, 2, S], qk_dt, tag="x8", bufs=2 * C2, name=f"x{name}{c}")
                        nc.sync.dma_start(out=x_t, in_=x_dram[:, c, :, :])
                        xc.append(x_t)
                    for t in range(DT):
                        for qb in range(QB):
                            acc = psacc.tile([128, 512], f32, tag="acc",
                                             name=f"pa{name}{t}{qb}")
                            for c in range(C2):
                                if QK_FP8:
                                    nc.tensor.matmul(
                                        acc,
                                        w_sb[:, c, :, t * 128:(t + 1) * 128],
                                        xc[c][:, :, qb * 512:(qb + 1) * 512],
                                        start=(c == 0), stop=(c == C2 - 1),
                                        perf_mode=DR,
                                    )
                                else:
                                    for o in range(2):
                                        nc.tensor.matmul(
                                            acc,
                                            w_sb[:, c, o, t * 128:(t + 1) * 128],
                                            xc[c][:, o, qb * 512:(qb + 1) * 512],
                                            start=(c == 0 and o == 0),
                                            stop=(c == C2 - 1 and o == 1),
                                        )
                            nc.vector.tensor_scalar_add(
                                o_tiles[t][:, qb * 512:(qb + 1) * 512],
                                acc,
                                b_sb[:, t:t + 1],
                            )

            # ---- attention, per head-pair and q-block ----
            with tc.tile_pool(name="attn", bufs=1) as attn:
                wo_sb = attn.tile([128, DT, D], bf16, tag="wo", bufs=1)
                nc.sync.dma_start(out=wo_sb, in_=wo_in[:, :, :])
                cn_sb = [attn.tile([128, S], bf16, tag="cn", bufs=DT, name=f"cn{h}") for h in range(DT)]
                if NOEXP:
                    ec = attn.tile([128, 1024], bf16, tag="ec", bufs=1)
                    nc.vector.memset(ec, 1.0)

                # unnormalized context (+denominator row 64) parked in SBUF so
                # the PSUM acc pair frees fast -> sct can triple-buffer
                cu_sb = [
                    attn.tile([65, 512], bf16, tag="cu", bufs=2 * DT * QB, name=f"cu{i}")
                    for i in range(2 * DT * QB)
                ]

                blocks = [(hp, qb) for hp in range(DT) for qb in range(QB)]
                NB = len(blocks) if KTX else 0
                NG = NB * KTX
                LOOK = 2
                bstate = {}

                def emit_sc(g):
                    hp, qb = blocks[g // KTX]
                    k = g % KTX
                    sct = pssc.tile([128, 1024], f32, tag="sc", name=f"sct{g}")
                    nc.tensor.matmul(
                        sct[:, 0:512],
                        kt_sb[hp][0:64, k * 128:(k + 1) * 128],
                        qt_sb[hp][0:64, qb * 512:(qb + 1) * 512],
                        start=True, stop=True, tile_position=(0, 0),
                    )
                    nc.tensor.matmul(
                        sct[:, 512:1024],
                        kt_sb[hp][64:128, k * 128:(k + 1) * 128],
                        qt_sb[hp][64:128, qb * 512:(qb + 1) * 512],
                        start=True, stop=True, tile_position=(64, 0),
                    )
                    if NOEXP:
                        et = ec
                        if NOEXP >= 2:
                            src_a = sct[:, 0:XACT] if NOEXP == 2 else ec[:, 0:XACT]
                            src_d = sct[:, XACT:1024] if NOEXP == 2 else bvr_sb[:, 0:1024 - XACT]
                            dt_ = attn.tile([128, 1024], bf16, tag="et", bufs=8, name=f"dt{g}")
                            nc.scalar.activation(
                                out=dt_[:, 0:XACT], in_=src_a, func=EXP,
                                bias=mb_sb[:, k:k + 1], scale=0.125,
                            )
                            nc.vector.tensor_scalar(
                                out=dt_[:, XACT:1024].bitcast(i16), in0=src_d,
                                scalar1=float(SCH_A * 0.125),
                                scalar2=mbs_sb[:, k:k + 1], op0=MULT, op1=ADD,
                            )
                    else:
                        et = attn.tile([128, 1024], bf16, tag="et", bufs=8, name=f"et{g}")
                        nc.scalar.activation(
                            out=et[:, 0:XACT], in_=sct[:, 0:XACT], func=EXP,
                            bias=mb_sb[:, k:k + 1], scale=0.125,
                        )
                        if XACT < 1024:
                            nc.vector.tensor_scalar(
                                out=et[:, XACT:1024].bitcast(i16),
                                in0=sct[:, XACT:1024],
                                scalar1=float(SCH_A * 0.125),
                                scalar2=mbs_sb[:, k:k + 1], op0=MULT, op1=ADD,
                            )
                    bstate[g] = et

                def emit_ctx(g):
                    bi = g // KTX
                    hp, qb = blocks[bi]
                    k = g % KTX
                    if k == 0:
                        bstate["acc", bi] = (
                            psacc.tile([65, 512], f32, tag="acc", name=f"ca0_{bi}"),
                            psacc.tile([65, 512], f32, tag="acc", name=f"ca1_{bi}"),
                        )
                    acc0, acc1 = bstate["acc", bi]
                    et = bstate.pop(g)
                    lh0, lh1 = 2 * hp, 2 * hp + 1
                    nc.tensor.matmul(
                        acc0, v_sb[k][:, lh0 * 65:lh0 * 65 + 65], et[:, 0:512],
                        start=(k == 0), stop=(k == KTX - 1),
                    )
                    nc.tensor.matmul(
                        acc1, v_sb[k][:, lh1 * 65:lh1 * 65 + 65], et[:, 512:1024],
                        start=(k == 0), stop=(k == KTX - 1),
                    )
                    if k == KTX - 1:
                        # park unnormalized ctx in SBUF: h0 copy on ACT, h1 on DVE
                        nc.scalar.copy(cu_sb[2 * bi], acc0)
                        nc.vector.tensor_copy(cu_sb[2 * bi + 1], acc1)

                for g in range(NG + LOOK):
                    # at block boundaries, retire the block's last ctx before
                    # issuing the next scores so the acc-pair copies start
                    # (and free the PSUM pair) as early as possible
                    late = g >= LOOK and (g - LOOK) % KTX == KTX - 1
                    if late:
                        emit_ctx(g - LOOK)
                    if g < NG:
                        emit_sc(g)
                    if g >= LOOK and not late:
                        emit_ctx(g - LOOK)

                # ---- normalize (pipelined ahead of output projection) ----
                def emit_norm(bi):
                    hp, qb = blocks[bi]
                    for half in range(2):
                        idx = 2 * bi + half
                        cu = cu_sb[idx]
                        r1 = attn.tile([128, 512], bf16, tag="r1", bufs=4, name=f"r1_{idx}")
                        with nc.allow_low_precision(reason="softmax denom recip in bf16"):
                            nc.vector.reciprocal(r1[64:65, :], cu[64:65, :])
                        nc.sync.dma_start(out=rscr[idx, :], in_=r1[64:65, :])
                        rr = attn.tile([64, 512], bf16, tag="rr", bufs=4, name=f"rr_{idx}")
                        nc.sync.dma_start(
                            out=rr, in_=rscr[idx, :].unsqueeze(0).partition_broadcast(64)
                        )
                        if half == 0:
                            nc.vector.tensor_mul(
                                cn_sb[hp][0:64, qb * 512:(qb + 1) * 512], cu[0:64, :], rr
                            )
                        else:
                            tm = attn.tile([64, 512], bf16, tag="tm", bufs=4, name=f"tm_{idx}")
                            nc.vector.tensor_mul(tm, cu[0:64, :], rr)
                            nc.sync.dma_start(
                                out=cn_sb[hp][64:128, qb * 512:(qb + 1) * 512], in_=tm
                            )

                # ---- output projection, qb-pipelined after its normalizes ----
                if not NOOUT and KTX:
                    for qb in range(QB + 1):
                        if qb < QB:
                            for hp in range(DT):
                                emit_norm(hp * QB + qb)
                        if qb == 0:
                            continue
                        for qt_i in range(4 * (qb - 1), 4 * qb):
                            pos = [
                                psacc.tile([128, 512], f32, tag="acc", name=f"po{qt_i}{nb}")
                                for nb in range(2)
                            ]
                            for hp in range(DT):
                                for nb in range(2):
                                    nc.tensor.matmul(
                                        pos[nb],
                                        cn_sb[hp][:, qt_i * 128:(qt_i + 1) * 128],
                                        wo_sb[:, hp, nb * 512:(nb + 1) * 512],
                                        start=(hp == 0), stop=(hp == DT - 1),
                                    )
                            for nb in range(2):
                                os_t = attn.tile([128, 512], bf16, tag="os", bufs=4, name=f"os{qt_i}{nb}")
                                nc.scalar.copy(os_t, pos[nb])
                                nc.sync.dma_start(
                                    out=out_d[qt_i * 128:(qt_i + 1) * 128, nb * 512:(nb + 1) * 512],
                                    in_=os_t,
                                )

    nc.finalize()
    return nc


def _build_dma_nc(niter=1):
    import concourse.mybir as mybir
    from concourse import bacc
    from concourse.tile import TileContext
    from contextlib import nullcontext

    f32 = mybir.dt.float32
    bf16 = mybir.dt.bfloat16
    fp8 = mybir.dt.float8e4
    qk_dt = fp8 if QK_FP8 else bf16

    nc = bacc.Bacc(None, target_bir_lowering=False)
    xq_in = nc.declare_dram_parameter("xq", [128, C2, 2, S], qk_dt, isOutput=False)
    xk_in = nc.declare_dram_parameter("xk", [128, C2, 2, S], qk_dt, isOutput=False)
    vt_in = nc.declare_dram_parameter("vt", [128, FCH, S], bf16, isOutput=False)
    wq_in = nc.declare_dram_parameter("wq", [128, C2, 2, DS], qk_dt, isOutput=False)
    wk_in = nc.declare_dram_parameter("wk", [128, C2, 2, DS], qk_dt, isOutput=False)
    wv_in = nc.declare_dram_parameter("wv", [128, FCH, 520], bf16, isOutput=False)
    wo_in = nc.declare_dram_parameter("wo", [128, DT, D], bf16, isOutput=False)
    out_d = nc.declare_dram_parameter("out", [S, D], bf16, isOutput=True)

    with TileContext(nc) as tc:
        with (
            tc.For_i(0, niter, 1) if niter > 1 else nullcontext(),
            tc.tile_pool(name="keep", bufs=1) as keep,
            tc.tile_pool(name="dps", bufs=1, space="PSUM") as dps,
        ):
            big = keep.tile([128, C2, 2, S], qk_dt, tag="b8", bufs=2, name="bg1")
            big2 = keep.tile([128, C2, 2, S], qk_dt, tag="b8", bufs=2, name="bg2")
            vtl = keep.tile([128, FCH, S], bf16)
            wvl = keep.tile([128, FCH, 520], bf16)
            wol = keep.tile([128, DT, D], bf16)
            wql = keep.tile([128, C2, 2, DS], qk_dt, tag="w8", bufs=2, name="wg1")
            wkl = keep.tile([128, C2, 2, DS], qk_dt, tag="w8", bufs=2, name="wg2")
            nc.sync.dma_start(out=big, in_=xq_in[:, :, :, :])
            nc.sync.dma_start(out=big2, in_=xk_in[:, :, :, :])
            nc.sync.dma_start(out=vtl, in_=vt_in[:, :, :])
            nc.sync.dma_start(out=wvl, in_=wv_in[:, :, :])
            nc.sync.dma_start(out=wol, in_=wo_in[:, :, :])
            nc.sync.dma_start(out=wql, in_=wq_in[:, :, :, :])
            nc.sync.dma_start(out=wkl, in_=wk_in[:, :, :, :])
            pt = dps.tile([128, 512], f32)
            nc.tensor.matmul(pt, vtl[:, 0, 0:128], wvl[:, 0, 0:512],
                             start=True, stop=True)
            nc.tensor.matmul(pt, big.bitcast(bf16)[:, 0, 0, 0:128],
                             big2.bitcast(bf16)[:, 0, 0, 0:512],
                             start=True, stop=True)
            nc.tensor.matmul(pt, wql.bitcast(bf16)[:, 0, 0, 0:128],
                             wkl.bitcast(bf16)[:, 0, 0, 0:512],
                             start=True, stop=True)
            nc.tensor.matmul(pt, wol[:, 0, 0:128], wol[:, 0, 0:512],
                             start=True, stop=True)
            ot = keep.tile([128, 512], bf16)
            nc.vector.tensor_copy(ot, pt)
            nc.sync.dma_start(out=out_d[0:128, 0:512], in_=ot)
    nc.finalize()
    return nc


def _get_nc(niter=1):
    key = ("nc", niter, DMAONLY)
    if key not in _cache:
        _cache[key] = (_build_dma_nc if DMAONLY else _build_nc)(niter)
    return _cache[key]


def _make_in_maps(query, key, value, mask, Wq, bq, Wk, bk, Wv, bv, Wo, bo):
    import ml_dtypes

    f = np.float32
    bf = ml_dtypes.bfloat16
    f8 = ml_dtypes.float8_e4m3fn
    in_maps = []
    for c in range(NCORES):
        b, hg = c // 2, c % 2
        hs = hg * DS
        wv_aug = np.zeros((D, 520), f)
        bvr_row = np.zeros((520,), f)
        for lh in range(8):
            wv_aug[:, lh * 65:lh * 65 + 64] = Wv[:, hs + lh * 64: hs + (lh + 1) * 64]
            bvr_row[lh * 65:lh * 65 + 64] = bv[hs + lh * 64: hs + (lh + 1) * 64]
            bvr_row[lh * 65 + 64] = 1.0
        mbias = np.where(mask[b, 0, 0, :] == 0, f(-1e9), f(0.0)).astype(f)

        qk_np = f8 if QK_FP8 else bf

        def dr_x(x):  # [S, D] -> [128, C2, 2, S] pairs
            xt = np.ascontiguousarray(x.T, dtype=f)           # [D, S]
            return np.ascontiguousarray(
                xt.reshape(C2, 2, 128, S).transpose(2, 0, 1, 3)
            ).astype(qk_np)

        def dr_w(W):  # [D, DS slice] -> [128, C2, 2, DS] pairs
            Ws = np.ascontiguousarray(W[:, hs:hs + DS], dtype=f)
            return np.ascontiguousarray(
                Ws.reshape(C2, 2, 128, DS).transpose(2, 0, 1, 3)
            ).astype(qk_np)

        vt = np.ascontiguousarray(value[b].T, dtype=f)        # [D, S]
        in_maps.append({
            "xq": dr_x(query[b]),
            "xk": dr_x(key[b]),
            "vt": np.ascontiguousarray(vt.reshape(FCH, 128, S).transpose(1, 0, 2)).astype(bf),
            "wq": dr_w(Wq),
            "wk": dr_w(Wk),
            "wv": np.ascontiguousarray(
                wv_aug.reshape(FCH, 128, 520).transpose(1, 0, 2)
            ).astype(bf),
            "wo": np.ascontiguousarray(
                np.asarray(Wo, f)[hs:hs + DS, :].reshape(DT, 128, D).transpose(1, 0, 2)
            ).astype(bf),
            "bq": np.ascontiguousarray(np.asarray(bq, f)[hs:hs + DS].reshape(DT, 128).T),
            "bk": np.ascontiguousarray(np.asarray(bk, f)[hs:hs + DS].reshape(DT, 128).T),
            "bvr": np.tile(bvr_row[None, :], (128, 1)).astype(f),
            "mb": np.ascontiguousarray(mbias.reshape(KT, 128).T, dtype=f),
            "mbs": np.ascontiguousarray(
                (mbias * f(SCH_A) + f(SCH_B)).reshape(KT, 128).T, dtype=f
            ),
        })
    return in_maps


def kernel(query, key, value, mask, Wq, bq, Wk, bk, Wv, bv, Wo, bo):
    from concourse.bass_utils import run_bass_kernel_spmd

    args = [np.asarray(a) for a in (query, key, value, mask, Wq, bq, Wk, bk, Wv, bv, Wo, bo)]
    query, key, value, mask, Wq, bq, Wk, bk, Wv, bv, Wo, bo = args
    nc = _get_nc()
    in_maps = _make_in_maps(query, key, value, mask, Wq, bq, Wk, bk, Wv, bv, Wo, bo)
    res = run_bass_kernel_spmd(nc, in_maps, list(range(NCORES)))
    out = np.empty((B, S, D), np.float32)
    for b in range(B):
        out[b] = (
            res.results[2 * b]["out"].astype(np.float32)
            + res.results[2 * b + 1]["out"].astype(np.float32)
            + bo[None, :]
        )
    return out
